# revision 3
# baseline (speedup 1.0000x reference)
"""Bidirectional cost volume kernel for Trainium2 (8 NeuronCores).

out[b, i+4, h, x] = mean_c f1[b,c,h,x] * f2[b,c,h,x-i],  i in [-4,4],
zero where x-i is out of range.  Inputs (8, 96, 96, 320) f32.

Sharding: data-parallel over batch, 1 sample per core.

Per-core algorithm, per image row h:
  - load f1row [96c, 320x], f2row [96c, 320x] to SBUF
  - 10 TensorE matmuls build band-Gram tiles stacked 32-deep in PSUM:
    chunk (t, j) covers x in [128t+32j ..+32); ptile[32j+r, 40t+c] =
    sum_c' f1[c', X+r] * f2[c', strip_start(t,j) + c]   (strip ~ X-4..X+36,
    clamped into [0, 320) for the two edge chunks)
  - the 9 outputs per x form a diagonal band of the Gram tile (offset
    varies with partition), which no compute engine can address directly;
    extract via 9 precomputed {0, 1/96} masks: one DVE tensor_mul with an
    s-replicated (zero-stride) read of the PSUM tile, then a segmented
    tensor_reduce over the 40-wide windows -> A[p, h*64 + t*16 + s].
  - masks fold in the 1/96 mean, the plane ordering, edge-chunk strip
    clamps, and zeroing of the never-written PSUM corner (t=2, p>=64).
After all rows: one DVE 32x32 block-transpose moves x from partitions to
the free axis; 3 DMAs then write out[plane, h, x] with 128B granules.
"""

import sys

sys.path.insert(0, "/opt/trn_rl_repo")

import numpy as np

import concourse.bass as bass
import concourse.tile as tile
from concourse import bacc, mybir
from concourse import bass_utils

BS, C, H, W = 8, 96, 96, 320
NS = 9          # shift planes
NT = 3          # 128-wide x stacks (last holds 64)
NCORES = 8
STRIP = 40


def _strip_start(t, j):
    X = 128 * t + 32 * j
    return min(max(X - 4, 0), W - STRIP)


def build_masks() -> np.ndarray:
    m = np.zeros((128, NS, NT * STRIP), np.float32)
    for p in range(128):
        j, r = p // 32, p % 32
        for t in range(NT):
            if t == 2 and j >= 2:
                continue  # PSUM corner never written: mask stays 0
            X = 128 * t + 32 * j
            x = X + r
            st = _strip_start(t, j)
            for plane in range(NS):
                i = plane - 4
                xp = x - i               # f2 column needed
                if xp < 0 or xp >= W:
                    continue             # reference zero-pads here
                c = xp - st
                if 0 <= c < STRIP:
                    m[p, plane, t * STRIP + c] = 1.0 / C
    return m


def cost_volume_kernel(ctx, tc, outs, ins):
    nc = tc.nc
    f1d, f2d, maskd = ins
    outd = outs[0]

    sb = ctx.enter_context(tc.tile_pool(name="sb", bufs=3))
    spool = ctx.enter_context(tc.tile_pool(name="scr", bufs=2))
    ppool = ctx.enter_context(tc.tile_pool(name="ps", bufs=4, space="PSUM"))
    fixed = ctx.enter_context(tc.tile_pool(name="fixed", bufs=1))

    masks = fixed.tile([128, NS * NT * STRIP], mybir.dt.float32)
    nc.sync.dma_start(masks[:], maskd[:])
    zeros = fixed.tile([128, STRIP], mybir.dt.float32)
    nc.gpsimd.memset(zeros[:], 0.0)
    A = fixed.tile([128, H * 64], mybir.dt.float32)
    nc.gpsimd.memset(A[:], 0.0)
    AT = fixed.tile([128, H * 64], mybir.dt.float32)

    mask4 = masks[:].rearrange("p (s t c) -> p s t c", s=NS, t=NT, c=STRIP)

    for h in range(H):
        f1row = sb.tile([C, W], mybir.dt.float32, tag="f1row")
        nc.sync.dma_start(f1row[:], f1d[:, h, :])
        f2row = sb.tile([C, W], mybir.dt.float32, tag="f2row")
        nc.sync.dma_start(f2row[:], f2d[:, h, :])

        ptile = ppool.tile([128, NT * STRIP], mybir.dt.float32)
        # matmuls never write [64:, 80:120]; keep it zero for the masked read
        nc.scalar.copy(ptile[64:128, 2 * STRIP:3 * STRIP], zeros[64:128, :])
        for t in range(NT):
            for j in range(4 if t < 2 else 2):
                X = 128 * t + 32 * j
                st = _strip_start(t, j)
                nc.tensor.matmul(
                    ptile[32 * j:32 * j + 32, t * STRIP:(t + 1) * STRIP],
                    f1row[:, X:X + 32],
                    f2row[:, st:st + STRIP],
                    start=True, stop=True,
                    tile_position=(0, 32 * j),
                )

        scr = spool.tile([128, NS * NT * STRIP], mybir.dt.float32)
        g_rep = (ptile[:].rearrange("p (t c) -> p t c", t=NT, c=STRIP)
                 .unsqueeze(1).broadcast_to((128, NS, NT, STRIP)))
        scr4 = scr[:].rearrange("p (s t c) -> p s t c", s=NS, t=NT, c=STRIP)
        nc.vector.tensor_mul(scr4, g_rep, mask4)
        # A[p, h*64 + t*16 + s] = sum_c scr[p, s, t, c]
        a_dst = (A[:, h * 64:h * 64 + 48]
                 .rearrange("p (t s) -> p s t", t=NT, s=16)[:, 0:NS, :])
        nc.vector.tensor_reduce(
            a_dst, scr4, axis=mybir.AxisListType.X, op=mybir.AluOpType.add,
        )

    # AT[32a+u, 32B+v] = A[32a+v, 32B+u]
    nc.vector.transpose(AT[:], A[:])

    # AT[32a + 16*tl + s, h*64 + 32*half + v] = out(plane s, h, x),
    # x = 128*(2*half + tl) + 32a + v
    at6 = AT[:].rearrange("(a tl s) (h half v) -> a tl s h half v",
                          a=4, tl=2, s=16, h=H, half=2, v=32)
    for t in range(NT):
        half, tl = t // 2, t % 2
        for a in range(4 if t < 2 else 2):
            src = at6[a, tl, 0:NS, :, half, :]           # (s, h, v)
            x0 = 128 * t + 32 * a
            dst = (outd[:, :, x0:x0 + 32]
                   .rearrange("s h v -> s h v"))
            nc.sync.dma_start(dst, src)


_CACHED = {}


def _build_program():
    if "nc" in _CACHED:
        return _CACHED["nc"]
    from contextlib import ExitStack
    nc = bacc.Bacc("TRN2", target_bir_lowering=False, debug=False,
                   enable_asserts=False, num_devices=NCORES)
    f1d = nc.dram_tensor("f1", [C, H, W], mybir.dt.float32,
                         kind="ExternalInput").ap()
    f2d = nc.dram_tensor("f2", [C, H, W], mybir.dt.float32,
                         kind="ExternalInput").ap()
    maskd = nc.dram_tensor("masks", [128, NS * NT * STRIP], mybir.dt.float32,
                           kind="ExternalInput").ap()
    outd = nc.dram_tensor("out", [NS, H, W], mybir.dt.float32,
                          kind="ExternalOutput").ap()
    with tile.TileContext(nc) as tc:
        with ExitStack() as ctx:
            cost_volume_kernel(ctx, tc, [outd], [f1d, f2d, maskd])
    nc.compile()
    _CACHED["nc"] = nc
    return nc


def make_in_maps(f1, f2):
    masks = build_masks()
    return [
        {"f1": np.ascontiguousarray(f1[b]),
         "f2": np.ascontiguousarray(f2[b]),
         "masks": masks}
        for b in range(BS)
    ]


def kernel(features_1: np.ndarray, features_2: np.ndarray) -> np.ndarray:
    f1 = np.asarray(features_1, np.float32)
    f2 = np.asarray(features_2, np.float32)
    assert f1.shape == (BS, C, H, W), f1.shape
    nc = _build_program()
    res = bass_utils.run_bass_kernel_spmd(nc, make_in_maps(f1, f2),
                                          list(range(NCORES)))
    return np.stack([res.results[b]["out"] for b in range(BS)], axis=0)


if __name__ == "__main__":
    rng = np.random.default_rng(0)
    f1 = rng.normal(size=(BS, C, H, W)).astype(np.float32)
    f2 = rng.normal(size=(BS, C, H, W)).astype(np.float32)
    out = kernel(f1, f2)
    print("out", out.shape, out.dtype, float(np.abs(out).mean()))
